# revision 4
# baseline (speedup 1.0000x reference)
"""Trainium2 Bass kernel for nn_CutoffModule (CBAM-style channel gate + topk gather).

Reference computation (per sample):
    avg/max spatial pooling -> shared 2-layer MLP -> sum -> sigmoid -> attn [C, D]
    per scale d: top-128 channels (sorted desc) -> gather those channels of x.

Sharding: data-parallel over N across 8 cores (4 samples/core); MLP weights
replicated. Entirely self-contained: hardcodes N=32, C=512, H=W=64, D=4, r=16.

v2 strategy (vs the gather baseline): x is read from HBM exactly once and kept
in SBUF; instead of re-reading x through an indirect gather, the kernel computes
the INVERSE permutation (channel -> output row, OOB sentinel when unselected)
and scatters x tiles straight to the output with indirect_dma_start
(out_offset + bounds_check skip). HBM traffic drops from 96 MiB to 64 MiB/core.

Notes:
- sigmoid is strictly monotonic, so top_k(sigmoid(y)) == top_k(y); the kernel
  ranks pre-sigmoid logits.
- topk row (d, n) lives on SBUF partition 32*d + n (engine writes must start
  at partition multiples of 32).
- inverse permutation per (sample, scale): one-hot M[k, c] = (topk_k == c) via
  tensor_scalar is_equal, then rank/selected via matmul with [iota128, ones];
  unselected channels get an out-of-bounds row id and are skipped by the DMA.
- engine plan: sync = all x loads; ACT = weights + avg pooling; Pool = pair-0
  max pooling then all scatters; DVE = topk/inverse + pair-1 max pooling
  (emitted after pair-0's inverse so the scatter never deadlocks on the
  x-buffer pool); PE = MLP + transposes + inverse matmuls.
"""

import numpy as np

import concourse.bacc as bacc
import concourse.bass as bass
import concourse.tile as tile
from concourse import mybir
from concourse.bass_utils import run_bass_kernel_spmd

# Problem constants (hardcoded per harness contract)
N_FULL = 32
C = 512
HW = 64 * 64          # 4096
D = 4                 # depth scales
BLOCK = C // D        # 128
HID = C // 16         # 32  (MLP hidden)
N_CORES = 8
NS = N_FULL // N_CORES  # 4 samples per core
P = 128               # SBUF partitions
CT = C // P           # 4 channel tiles per sample
NEG_FILL = -1e30
BIG = 3000.0          # OOB sentinel offset base (> NS*C-1)
XBUFS = 10            # x tile buffers (16 KiB/partition each)

F32 = mybir.dt.float32
U32 = mybir.dt.uint32


def _build_program():
    nc = bacc.Bacc("TRN2", target_bir_lowering=False, debug=False)

    x_d = nc.dram_tensor("x", [NS * C, HW], F32, kind="ExternalInput").ap()
    w1_d = nc.dram_tensor("w1", [C, HID], F32, kind="ExternalInput").ap()
    b1_d = nc.dram_tensor("b1", [HID, 1], F32, kind="ExternalInput").ap()
    # w2aug = [W2; 2*b2] so layer 2 + both bias adds fold into one K=33 matmul
    w2_d = nc.dram_tensor("w2aug", [HID + 1, C * D], F32, kind="ExternalInput").ap()
    ident_d = nc.dram_tensor("ident", [P, P], F32, kind="ExternalInput").ap()
    iota_d = nc.dram_tensor("iota512", [P, C], F32, kind="ExternalInput").ap()
    rhs2_d = nc.dram_tensor("rhs2", [P, 2], F32, kind="ExternalInput").ap()
    out_d = nc.dram_tensor("out", [NS * C, HW], F32, kind="ExternalOutput").ap()

    with tile.TileContext(nc) as tc:
        with (
            tc.tile_pool(name="xin", bufs=XBUFS) as xpool,
            tc.tile_pool(name="small", bufs=1) as sm,
            tc.tile_pool(name="mbuf", bufs=2) as mpool,
            tc.tile_pool(name="psum", bufs=1, space="PSUM") as psum,
            tc.tile_pool(name="psumr", bufs=2, space="PSUM") as psumr,
        ):
            # ---- constants / weights into SBUF on the scalar(ACT) ring ----
            w1_sb = sm.tile([P, CT, HID], F32)   # chunk ct = channels ct*128..+128
            nc.scalar.dma_start(
                out=w1_sb[:], in_=w1_d.rearrange("(c p) m -> p c m", p=P)
            )
            w2_sb = sm.tile([HID + 1, C * D], F32)
            nc.scalar.dma_start(out=w2_sb[:], in_=w2_d)
            b1_sb = sm.tile([HID, 1], F32)
            nc.scalar.dma_start(out=b1_sb[:], in_=b1_d)
            ident_sb = sm.tile([P, P], F32)
            nc.scalar.dma_start(out=ident_sb[:], in_=ident_d)
            iota_sb = sm.tile([P, C], F32)
            nc.scalar.dma_start(out=iota_sb[:], in_=iota_d)
            rhs2_sb = sm.tile([P, 2], F32)
            nc.scalar.dma_start(out=rhs2_sb[:], in_=rhs2_d)

            # pooling accumulators: [P, ct, {avg0, avg1, max0, max1}] per pair
            pools = [sm.tile([P, CT, 4], F32, name=f"pools{pp}") for pp in range(2)]
            scratch = sm.tile([P, HW], F32)

            vals = [[sm.tile([P, C], F32, name=f"vals{pp}_{i}") for i in range(2)]
                    for pp in range(2)]
            for pp in range(2):
                nc.vector.memset(vals[pp][0][:], 0.0)

            # per-pair scatter offsets: [P, (d, i), ct] u32
            offs_u = [sm.tile([P, 2 * D, CT], U32, name=f"offs{pp}")
                      for pp in range(2)]

            xtiles = {}

            def load_and_avg(n):
                """Loads on sync ring; avg pooling on ACT (chases the loads)."""
                pp, i = divmod(n, 2)
                for ct in range(CT):
                    row0 = (n * CT + ct) * P
                    xt = xpool.tile([P, HW], F32, tag="xt")
                    xtiles[(n, ct)] = xt
                    nc.sync.dma_start(out=xt[:], in_=x_d[row0 : row0 + P, :])
                    nc.scalar.activation(
                        out=scratch[:],
                        in_=xt[:],
                        func=mybir.ActivationFunctionType.Copy,
                        scale=1.0 / HW,
                        accum_out=pools[pp][:, ct, i : i + 1],
                    )

            def max_pool(n):
                pp, i = divmod(n, 2)
                for ct in range(CT):
                    nc.vector.reduce_max(
                        out=pools[pp][:, ct, 2 + i : 3 + i],
                        in_=xtiles[(n, ct)][:],
                        axis=mybir.AxisListType.X,
                    )

            def mlp_pair(pp):
                """MLP for samples {2pp, 2pp+1}: py rows 32d+i = y[2pp+i, :]."""
                ph = psum.tile([HID, 4], F32, space="PSUM", tag="ph")
                for ct in range(CT):
                    nc.tensor.matmul(
                        out=ph[:],
                        lhsT=w1_sb[:, ct, :],
                        rhs=pools[pp][:, ct, :],
                        start=(ct == 0),
                        stop=(ct == CT - 1),
                    )
                hTa = sm.tile([HID, 2], F32, name=f"hTa{pp}")
                hTm = sm.tile([HID, 2], F32, name=f"hTm{pp}")
                nc.scalar.activation(
                    out=hTa[:], in_=ph[:, 0:2],
                    func=mybir.ActivationFunctionType.Relu, bias=b1_sb[:, :],
                )
                nc.scalar.activation(
                    out=hTm[:], in_=ph[:, 2:4],
                    func=mybir.ActivationFunctionType.Relu, bias=b1_sb[:, :],
                )
                hsum = sm.tile([HID, 2], F32, name=f"hsum{pp}")
                nc.vector.tensor_add(out=hsum[:], in0=hTa[:], in1=hTm[:])
                # augmented lhsT: rows 0-31 = hsum replicated at cols 32d+i,
                # row 32 = 1.0 (bias row of w2aug)
                hw_t = sm.tile([HID + 1, P], F32, name=f"hw{pp}")
                nc.vector.memset(hw_t[:], 0.0)
                nc.vector.memset(hw_t[32:33, :], 1.0)
                for d in range(D):
                    nc.vector.tensor_copy(
                        out=hw_t[0:HID, 32 * d : 32 * d + 2], in_=hsum[:]
                    )

                py = psum.tile([P, C * D], F32, space="PSUM", tag="py")
                for s in range(C * D // 512):
                    sl = slice(s * 512, (s + 1) * 512)
                    nc.tensor.matmul(
                        out=py[:, sl], lhsT=hw_t[:], rhs=w2_sb[:, sl],
                        start=True, stop=True,
                    )
                # vals[32d+i, c] = y[i, c*D + d]
                va = vals[pp][0]
                for d in range(D):
                    nc.vector.tensor_copy(
                        out=va[32 * d : 32 * d + 2, :],
                        in_=py[32 * d : 32 * d + 2, d :: D],
                    )

            def topk_pair(pp):
                """ptf column 32d+i = topk channel ids (rank k on partitions)."""
                topk_idx = sm.tile([P, BLOCK], U32, name=f"tki{pp}")
                maxv = sm.tile([P, 8], F32, name=f"maxv{pp}")
                cur, nxt = vals[pp]
                for k in range(BLOCK // 8):
                    nc.vector.max(out=maxv[:], in_=cur[:])
                    nc.vector.max_index(
                        out=topk_idx[:, 8 * k : 8 * k + 8],
                        in_max=maxv[:],
                        in_values=cur[:],
                    )
                    if k < BLOCK // 8 - 1:
                        nc.vector.match_replace(
                            out=nxt[:], in_to_replace=maxv[:], in_values=cur[:],
                            imm_value=NEG_FILL,
                        )
                        cur, nxt = nxt, cur

                idx_f = sm.tile([P, BLOCK], F32, name=f"idxf{pp}")
                nc.vector.tensor_copy(out=idx_f[:], in_=topk_idx[:])
                pt = psum.tile([P, P], F32, space="PSUM", tag="pt")
                nc.tensor.transpose(out=pt[:], in_=idx_f[:], identity=ident_sb[:])
                ptf = sm.tile([P, P], F32, name=f"ptf{pp}")
                nc.vector.tensor_copy(out=ptf[:], in_=pt[:])
                return ptf

            def inverse_pair(pp, ptf):
                """offs_u[:, 2d+i? no: (d,i) packed] = output row per channel."""
                for i in range(2):
                    for d in range(D):
                        col = 32 * d + i
                        j8 = 2 * d + i
                        base = float((2 * pp + i) * C + d * BLOCK)
                        m = mpool.tile([P, C], F32, tag="m")
                        nc.vector.tensor_scalar(
                            out=m[:], in0=iota_sb[:],
                            scalar1=ptf[:, col : col + 1], scalar2=None,
                            op0=mybir.AluOpType.is_equal,
                        )
                        psR = psumr.tile([P, CT, 2], F32, space="PSUM", tag="psr")
                        for ct in range(CT):
                            nc.tensor.matmul(
                                out=psR[:, ct, :],
                                lhsT=m[:, ct * P : (ct + 1) * P],
                                rhs=rhs2_sb[:],
                                start=True, stop=True,
                            )
                        # offs = rank*sel + (BIG+base) - BIG*sel (+base*sel)
                        tmp = sm.tile([P, CT], F32, name=f"tmp{pp}")
                        nc.vector.tensor_scalar(
                            out=tmp[:], in0=psR[:, :, 1],
                            scalar1=base - BIG, scalar2=BIG,
                            op0=mybir.AluOpType.mult,
                            op1=mybir.AluOpType.add,
                        )
                        offs_f = sm.tile([P, CT], F32, name=f"offsf{pp}")
                        nc.vector.tensor_add(
                            out=offs_f[:], in0=tmp[:], in1=psR[:, :, 0]
                        )
                        nc.vector.tensor_copy(
                            out=offs_u[pp][:, j8, :], in_=offs_f[:]
                        )

            def scatter_pair(pp):
                # (i, ct, d) order: tile (n, ct) retires after its 4th scatter
                for i in range(2):
                    n = 2 * pp + i
                    for ct in range(CT):
                        xt = xtiles[(n, ct)]
                        for d in range(D):
                            j8 = 2 * d + i
                            nc.gpsimd.indirect_dma_start(
                                out=out_d[:, :],
                                out_offset=bass.IndirectOffsetOnAxis(
                                    ap=offs_u[pp][:, j8, ct : ct + 1], axis=0
                                ),
                                in_=xt[:],
                                in_offset=None,
                                bounds_check=NS * C - 1,
                                oob_is_err=False,
                            )

            # ---- emission order == engine program order.  pair-1 avg pools
            # (ACT) must be emitted after pair-0's relus, and pair-1 max pools
            # (DVE) after pair-0's inverse, so no engine stalls on a gated
            # pair-1 x load before pair-0's scatter chain (which releases the
            # x buffers) is reachable.
            for n in (0, 1):
                load_and_avg(n)
            for n in (0, 1):
                max_pool(n)                  # DVE
            mlp_pair(0)
            ptf0 = topk_pair(0)
            inverse_pair(0, ptf0)
            for n in (2, 3):
                load_and_avg(n)              # sync ring stalls on x buffers, ok
            scatter_pair(0)                  # Pool
            for n in (2, 3):
                max_pool(n)                  # DVE, after inverse0 (no deadlock)
            mlp_pair(1)
            ptf1 = topk_pair(1)
            inverse_pair(1, ptf1)
            scatter_pair(1)

    nc.compile()
    return nc


_NC_CACHE = None


def _get_nc():
    global _NC_CACHE
    if _NC_CACHE is None:
        _NC_CACHE = _build_program()
    return _NC_CACHE


def _make_in_maps(x, W1, b1, W2, b2):
    x = np.ascontiguousarray(np.asarray(x, dtype=np.float32)).reshape(N_FULL, C, HW)
    W1 = np.asarray(W1, dtype=np.float32)
    b1 = np.asarray(b1, dtype=np.float32).reshape(HID, 1)
    W2 = np.asarray(W2, dtype=np.float32)
    b2 = np.asarray(b2, dtype=np.float32).reshape(1, C * D)
    w2aug = np.ascontiguousarray(np.vstack([W2, 2.0 * b2]))
    ident = np.eye(P, dtype=np.float32)
    iota512 = np.tile(np.arange(C, dtype=np.float32), (P, 1))
    rhs2 = np.stack(
        [np.arange(P, dtype=np.float32), np.ones(P, dtype=np.float32)], axis=1
    )
    rhs2 = np.ascontiguousarray(rhs2)
    in_maps = []
    for core in range(N_CORES):
        shard = x[core * NS : (core + 1) * NS].reshape(NS * C, HW)
        in_maps.append(
            {
                "x": np.ascontiguousarray(shard),
                "w1": W1,
                "b1": b1,
                "w2aug": w2aug,
                "ident": ident,
                "iota512": iota512,
                "rhs2": rhs2,
            }
        )
    return in_maps


def run(inputs, trace=False, **kwargs):
    """Run the SPMD kernel; returns (full_output, BassKernelResults)."""
    nc = _get_nc()
    in_maps = _make_in_maps(
        inputs["x"], inputs["W1"], inputs["b1"], inputs["W2"], inputs["b2"]
    )
    res = run_bass_kernel_spmd(
        nc, in_maps, core_ids=list(range(N_CORES)), trace=trace, **kwargs
    )
    parts = [res.results[i]["out"].reshape(NS, C, 64, 64) for i in range(N_CORES)]
    out = np.concatenate(parts, axis=0)
    return out, res


def kernel(**inputs) -> np.ndarray:
    out, _ = run(inputs)
    return out


# revision 9
# speedup vs baseline: 1.4668x; 1.4668x over previous
"""Trainium2 Bass kernel for nn_CutoffModule (CBAM-style channel gate + topk gather).

Reference computation (per sample):
    avg/max spatial pooling -> shared 2-layer MLP -> sum -> sigmoid -> attn [C, D]
    per scale d: top-128 channels (sorted desc) -> gather those channels of x.

Sharding: data-parallel over N across 8 cores (4 samples/core); MLP weights
replicated. Entirely self-contained: hardcodes N=32, C=512, H=W=64, D=4, r=16.

v2 strategy (vs the gather baseline): x is read from HBM exactly once and kept
in SBUF; instead of re-reading x through an indirect gather, the kernel computes
the INVERSE permutation (channel -> output row, OOB sentinel when unselected)
and scatters x tiles straight to the output with indirect_dma_start
(out_offset + bounds_check skip). HBM traffic drops from 96 MiB to 64 MiB/core.

Notes:
- sigmoid is strictly monotonic, so top_k(sigmoid(y)) == top_k(y); the kernel
  ranks pre-sigmoid logits.
- topk row (d, n) lives on SBUF partition 32*d + n (engine writes must start
  at partition multiples of 32).
- inverse permutation per (sample, scale): one-hot M[k, c] = (topk_k == c) via
  tensor_scalar is_equal, then rank/selected via matmul with [iota128, ones];
  unselected channels get an out-of-bounds row id and are skipped by the DMA.
- engine plan: sync = all x loads; ACT = weights + avg pooling; Pool = pair-0
  max pooling then all scatters; DVE = topk/inverse + pair-1 max pooling
  (emitted after pair-0's inverse so the scatter never deadlocks on the
  x-buffer pool); PE = MLP + transposes + inverse matmuls.
"""

import numpy as np

import concourse.bacc as bacc
import concourse.bass as bass
import concourse.tile as tile
from concourse import mybir
from concourse.bass_utils import run_bass_kernel_spmd

# Problem constants (hardcoded per harness contract)
N_FULL = 32
C = 512
HW = 64 * 64          # 4096
D = 4                 # depth scales
BLOCK = C // D        # 128
HID = C // 16         # 32  (MLP hidden)
N_CORES = 8
NS = N_FULL // N_CORES  # 4 samples per core
P = 128               # SBUF partitions
CT = C // P           # 4 channel tiles per sample
NEG_FILL = -1e30
BIG = 200.0           # OOB sentinel offset (> BLOCK-1)
XBUFS = 10            # x tile buffers (16 KiB/partition each)

F32 = mybir.dt.float32
U32 = mybir.dt.uint32


def _build_program():
    nc = bacc.Bacc("TRN2", target_bir_lowering=False, debug=False)

    x_d = nc.dram_tensor("x", [NS * C, HW], F32, kind="ExternalInput").ap()
    w1_d = nc.dram_tensor("w1", [C, HID], F32, kind="ExternalInput").ap()
    b1_d = nc.dram_tensor("b1", [HID, 1], F32, kind="ExternalInput").ap()
    # w2aug = [W2; 2*b2] so layer 2 + both bias adds fold into one K=33 matmul
    w2_d = nc.dram_tensor("w2aug", [HID + 1, C * D], F32, kind="ExternalInput").ap()
    ident_d = nc.dram_tensor("ident", [P, P], F32, kind="ExternalInput").ap()
    iota_d = nc.dram_tensor("iota512", [P, C], F32, kind="ExternalInput").ap()
    rhs2_d = nc.dram_tensor("rhs2", [P, 2], F32, kind="ExternalInput").ap()
    # one output tensor per (sample, scale) block: scatters to different
    # blocks are independent (a single shared output tensor makes the tile
    # framework chain every scatter on the previous one's completion)
    out_d = [
        [
            nc.dram_tensor(f"out_{n}_{d}", [BLOCK, HW], F32, kind="ExternalOutput").ap()
            for d in range(D)
        ]
        for n in range(NS)
    ]

    with tile.TileContext(nc) as tc:
        with (
            tc.tile_pool(name="xin", bufs=XBUFS) as xpool,
            tc.tile_pool(name="small", bufs=1) as sm,
            tc.tile_pool(name="mbuf", bufs=2) as mpool,
            tc.tile_pool(name="psum", bufs=1, space="PSUM") as psum,
            tc.tile_pool(name="psumr", bufs=2, space="PSUM") as psumr,
        ):
            # ---- constants / weights into SBUF on the scalar(ACT) ring ----
            w1_sb = sm.tile([P, CT, HID], F32)   # chunk ct = channels ct*128..+128
            nc.scalar.dma_start(
                out=w1_sb[:], in_=w1_d.rearrange("(c p) m -> p c m", p=P)
            )
            w2_sb = sm.tile([HID + 1, C * D], F32)
            nc.scalar.dma_start(out=w2_sb[:], in_=w2_d)
            b1_sb = sm.tile([HID, 1], F32)
            nc.scalar.dma_start(out=b1_sb[:], in_=b1_d)
            ident_sb = sm.tile([P, P], F32)
            nc.scalar.dma_start(out=ident_sb[:], in_=ident_d)
            iota_sb = sm.tile([P, C], F32)
            nc.scalar.dma_start(out=iota_sb[:], in_=iota_d)
            rhs2_sb = sm.tile([P, 2], F32)
            nc.scalar.dma_start(out=rhs2_sb[:], in_=rhs2_d)

            # pooling accumulators: [P, ct, {avg0, avg1, max0, max1}] per pair
            pools = [sm.tile([P, CT, 4], F32, name=f"pools{pp}") for pp in range(2)]
            scratch = sm.tile([P, HW], F32)

            vals = [[sm.tile([P, C], F32, name=f"vals{pp}_{i}") for i in range(2)]
                    for pp in range(2)]
            for pp in range(2):
                nc.vector.memset(vals[pp][0][:], 0.0)

            # per-pair scatter offsets: [P, (d, i), ct] u32
            offs_u = [sm.tile([P, 2 * D, CT], U32, name=f"offs{pp}")
                      for pp in range(2)]

            xtiles = {}

            def load_and_avg(n):
                """Loads on sync ring; avg pooling on ACT (chases the loads)."""
                pp, i = divmod(n, 2)
                for ct in range(CT):
                    row0 = (n * CT + ct) * P
                    xt = xpool.tile([P, HW], F32, tag="xt")
                    xtiles[(n, ct)] = xt
                    nc.sync.dma_start(out=xt[:], in_=x_d[row0 : row0 + P, :])
                    nc.scalar.activation(
                        out=scratch[:],
                        in_=xt[:],
                        func=mybir.ActivationFunctionType.Copy,
                        scale=1.0 / HW,
                        accum_out=pools[pp][:, ct, i : i + 1],
                    )

            def max_pool(n):
                pp, i = divmod(n, 2)
                for ct in range(CT):
                    nc.vector.reduce_max(
                        out=pools[pp][:, ct, 2 + i : 3 + i],
                        in_=xtiles[(n, ct)][:],
                        axis=mybir.AxisListType.X,
                    )

            def mlp_pair(pp):
                """MLP for samples {2pp, 2pp+1}: py rows 32d+i = y[2pp+i, :]."""
                ph = psum.tile([HID, 4], F32, space="PSUM", tag="ph")
                for ct in range(CT):
                    nc.tensor.matmul(
                        out=ph[:],
                        lhsT=w1_sb[:, ct, :],
                        rhs=pools[pp][:, ct, :],
                        start=(ct == 0),
                        stop=(ct == CT - 1),
                    )
                hTa = sm.tile([HID, 2], F32, name=f"hTa{pp}")
                hTm = sm.tile([HID, 2], F32, name=f"hTm{pp}")
                nc.scalar.activation(
                    out=hTa[:], in_=ph[:, 0:2],
                    func=mybir.ActivationFunctionType.Relu, bias=b1_sb[:, :],
                )
                nc.scalar.activation(
                    out=hTm[:], in_=ph[:, 2:4],
                    func=mybir.ActivationFunctionType.Relu, bias=b1_sb[:, :],
                )
                hsum = sm.tile([HID, 2], F32, name=f"hsum{pp}")
                nc.vector.tensor_add(out=hsum[:], in0=hTa[:], in1=hTm[:])
                # augmented lhsT: rows 0-31 = hsum replicated at cols 32d+i,
                # row 32 = 1.0 (bias row of w2aug)
                hw_t = sm.tile([HID + 1, P], F32, name=f"hw{pp}")
                nc.vector.memset(hw_t[:], 0.0)
                nc.vector.memset(hw_t[32:33, :], 1.0)
                for d in range(D):
                    nc.vector.tensor_copy(
                        out=hw_t[0:HID, 32 * d : 32 * d + 2], in_=hsum[:]
                    )

                py = psum.tile([P, C * D], F32, space="PSUM", tag="py")
                for s in range(C * D // 512):
                    sl = slice(s * 512, (s + 1) * 512)
                    nc.tensor.matmul(
                        out=py[:, sl], lhsT=hw_t[:], rhs=w2_sb[:, sl],
                        start=True, stop=True,
                    )
                # vals[32d+i, c] = y[i, c*D + d]
                va = vals[pp][0]
                for d in range(D):
                    nc.vector.tensor_copy(
                        out=va[32 * d : 32 * d + 2, :],
                        in_=py[32 * d : 32 * d + 2, d :: D],
                    )

            def topk_pair(pp):
                """ptf column 32d+i = topk channel ids (rank k on partitions)."""
                topk_idx = sm.tile([P, BLOCK], U32, name=f"tki{pp}")
                maxv = sm.tile([P, 8], F32, name=f"maxv{pp}")
                cur, nxt = vals[pp]
                for k in range(BLOCK // 8):
                    nc.vector.max(out=maxv[:], in_=cur[:])
                    nc.vector.max_index(
                        out=topk_idx[:, 8 * k : 8 * k + 8],
                        in_max=maxv[:],
                        in_values=cur[:],
                    )
                    if k < BLOCK // 8 - 1:
                        nc.vector.match_replace(
                            out=nxt[:], in_to_replace=maxv[:], in_values=cur[:],
                            imm_value=NEG_FILL,
                        )
                        cur, nxt = nxt, cur

                idx_f = sm.tile([P, BLOCK], F32, name=f"idxf{pp}")
                nc.vector.tensor_copy(out=idx_f[:], in_=topk_idx[:])
                pt = psum.tile([P, P], F32, space="PSUM", tag="pt")
                nc.tensor.transpose(out=pt[:], in_=idx_f[:], identity=ident_sb[:])
                ptf = sm.tile([P, P], F32, name=f"ptf{pp}")
                nc.vector.tensor_copy(out=ptf[:], in_=pt[:])
                return ptf

            def inverse_pair(pp, ptf):
                """offs_u[:, 2d+i? no: (d,i) packed] = output row per channel."""
                for i in range(2):
                    for d in range(D):
                        col = 32 * d + i
                        j8 = 2 * d + i
                        m = mpool.tile([P, C], F32, tag="m")
                        nc.vector.tensor_scalar(
                            out=m[:], in0=iota_sb[:],
                            scalar1=ptf[:, col : col + 1], scalar2=None,
                            op0=mybir.AluOpType.is_equal,
                        )
                        psR = psumr.tile([P, CT, 2], F32, space="PSUM", tag="psr")
                        for ct in range(CT):
                            nc.tensor.matmul(
                                out=psR[:, ct, :],
                                lhsT=m[:, ct * P : (ct + 1) * P],
                                rhs=rhs2_sb[:],
                                start=True, stop=True,
                            )
                        # offs = rank*sel + BIG*(1-sel)
                        tmp = sm.tile([P, CT], F32, name=f"tmp{pp}")
                        nc.vector.tensor_scalar(
                            out=tmp[:], in0=psR[:, :, 1],
                            scalar1=-BIG, scalar2=BIG,
                            op0=mybir.AluOpType.mult,
                            op1=mybir.AluOpType.add,
                        )
                        offs_f = sm.tile([P, CT], F32, name=f"offsf{pp}")
                        nc.vector.tensor_add(
                            out=offs_f[:], in0=tmp[:], in1=psR[:, :, 0]
                        )
                        nc.vector.tensor_copy(
                            out=offs_u[pp][:, j8, :], in_=offs_f[:]
                        )

            def scatter_pair(pp):
                # ct-major round-robin: consecutive scatters hit different
                # output tensors (no same-tensor wait), and tile (n, ct)
                # retires after one ct pass
                for ct in range(CT):
                    for i in range(2):
                        n = 2 * pp + i
                        xt = xtiles[(n, ct)]
                        for d in range(D):
                            j8 = 2 * d + i
                            nc.gpsimd.indirect_dma_start(
                                out=out_d[n][d][:, :],
                                out_offset=bass.IndirectOffsetOnAxis(
                                    ap=offs_u[pp][:, j8, ct : ct + 1], axis=0
                                ),
                                in_=xt[:],
                                in_offset=None,
                                bounds_check=BLOCK - 1,
                                oob_is_err=False,
                            )

            # ---- emission order == engine program order.  pair-1 avg pools
            # (ACT) must be emitted after pair-0's relus, and pair-1 max pools
            # (DVE) after pair-0's inverse, so no engine stalls on a gated
            # pair-1 x load before pair-0's scatter chain (which releases the
            # x buffers) is reachable.
            for n in (0, 1):
                load_and_avg(n)
            for n in (0, 1):
                max_pool(n)                  # DVE
            mlp_pair(0)
            ptf0 = topk_pair(0)
            inverse_pair(0, ptf0)
            for n in (2, 3):
                load_and_avg(n)              # sync ring stalls on x buffers, ok
            scatter_pair(0)                  # Pool
            for n in (2, 3):
                max_pool(n)                  # DVE, after inverse0 (no deadlock)
            mlp_pair(1)
            ptf1 = topk_pair(1)
            inverse_pair(1, ptf1)
            scatter_pair(1)

    nc.compile()
    return nc


_NC_CACHE = None


def _get_nc():
    global _NC_CACHE
    if _NC_CACHE is None:
        _NC_CACHE = _build_program()
    return _NC_CACHE


def _make_in_maps(x, W1, b1, W2, b2):
    x = np.ascontiguousarray(np.asarray(x, dtype=np.float32)).reshape(N_FULL, C, HW)
    W1 = np.asarray(W1, dtype=np.float32)
    b1 = np.asarray(b1, dtype=np.float32).reshape(HID, 1)
    W2 = np.asarray(W2, dtype=np.float32)
    b2 = np.asarray(b2, dtype=np.float32).reshape(1, C * D)
    w2aug = np.ascontiguousarray(np.vstack([W2, 2.0 * b2]))
    ident = np.eye(P, dtype=np.float32)
    iota512 = np.tile(np.arange(C, dtype=np.float32), (P, 1))
    rhs2 = np.stack(
        [np.arange(P, dtype=np.float32), np.ones(P, dtype=np.float32)], axis=1
    )
    rhs2 = np.ascontiguousarray(rhs2)
    in_maps = []
    for core in range(N_CORES):
        shard = x[core * NS : (core + 1) * NS].reshape(NS * C, HW)
        in_maps.append(
            {
                "x": np.ascontiguousarray(shard),
                "w1": W1,
                "b1": b1,
                "w2aug": w2aug,
                "ident": ident,
                "iota512": iota512,
                "rhs2": rhs2,
            }
        )
    return in_maps


def run(inputs, trace=False, **kwargs):
    """Run the SPMD kernel; returns (full_output, BassKernelResults)."""
    nc = _get_nc()
    in_maps = _make_in_maps(
        inputs["x"], inputs["W1"], inputs["b1"], inputs["W2"], inputs["b2"]
    )
    res = run_bass_kernel_spmd(
        nc, in_maps, core_ids=list(range(N_CORES)), trace=trace, **kwargs
    )
    out = np.empty((N_FULL, C, 64, 64), dtype=np.float32)
    for core in range(N_CORES):
        r = res.results[core]
        for n in range(NS):
            for d in range(D):
                blk = r[f"out_{n}_{d}"].reshape(BLOCK, 64, 64)
                out[core * NS + n, d * BLOCK : (d + 1) * BLOCK] = blk
    return out, res


def kernel(**inputs) -> np.ndarray:
    out, _ = run(inputs)
    return out
